# revision 7
# baseline (speedup 1.0000x reference)
"""Trainium2 Bass kernel for nn_Attention_36146444763783.

GroupNorm(32) + SiLU -> QKV proj -> 8-head attention (n=1024) -> out proj
+ bias + residual, batch=16, fully data-parallel: 2 batches per NeuronCore
across 8 cores.

Per-core dataflow (fp8 DoubleRow matmuls for the QKV projections, plain fp8
for logits + PV, bf16 for out proj; fp32 PSUM everywhere):
  - x [2,1024,512] fp32 loaded as [128, 8*512] tiles (partition = token%128)
  - GroupNorm stats per (batch, group) via DVE reduces (x^2 on Pool) + PE
    ones-matmul partition sums; per-channel affine A,B expanded to [128,4]
    via a selector matmul; normalize+SiLU runs on PE-transposed x blocks,
    final mul (Pool) writes fp8 xnT packed [128, 2(c-sub), 1024] so the QKV
    matmuls can run DoubleRow (contraction 512 done as 2x256)
  - wqkv host-scaled 16x into fp8 (so q,k,v = 16*raw); q,k stored [d, n]
    fp8, v stored [n, 8*65] fp8 with a sumexp ones column per head
  - logits: simT[j,i] = k^T q, plain fp8 matmuls (contraction 64);
    exp(logit) = exp(psum * 2^-11) via ScalarE activation-scale and a
    custom degree-3 DVE poly op, split by a static pattern; fp8 eT out
  - PV: plain fp8 matmuls (FD=65: attn-out + sumexp column), fp8 FWL keeps
    weight loads hidden; PSUM drained with a broadcast normalize
    (reciprocal of the sumexp column) into bf16 ao
  - out proj bf16 from PE-transposed ao with wout/16 host-folded; residual
    add on DVE, bias add on Pool
  - both batches' prologues are emitted before attention so the second
    batch's GroupNorm/QKV overlaps the first batch's attention
"""

import sys

import numpy as np

sys.path.insert(0, "/opt/trn_rl_repo")

B, HGT, WID, CH = 16, 32, 32, 512
HEADS, HEAD_CH, HIDDEN = 8, 64, 512
GROUPS = 32
EPS = 1e-5
N = HGT * WID  # 1024 tokens per batch
N_CORES = 8
BPC = B // N_CORES  # batches per core
NT = N // 128  # 8 token tiles
CC = CH // 128  # 4 channel chunks

W_SCALE = 16.0  # fp8 range scaling for wqkv (and q,k,v downstream)
EXP_SCALE = 1.0 / (W_SCALE * W_SCALE * (HEAD_CH**0.5))  # 2^-11

_EXP3 = None


def _register_exp3():
    """Degree-3 polynomial for exp(s*x) as a custom DVE op:
    1 + (sx) + (sx)^2/2 + (sx)^3/6 with the scale baked into the
    coefficients. Valid for |sx| <= ~0.5."""
    global _EXP3
    if _EXP3 is not None:
        return _EXP3
    from concourse import dve_ops
    from concourse.dve_spec import Spec, Src0, C0, C1, C2, One, lower
    from concourse.dve_uop import DveOpSpec

    name = "EXP3_ANT"
    if name not in dve_ops._SUB_OPCODE_FOR_NAME:
        body = ((Src0 * C0 + C1) * Src0 + C2) * Src0 + One
        spec = Spec(
            body=body,
            reference=lambda in0, in1, s0, s1, imm2: (
                ((in0 * s0 + s1) * in0 + imm2) * in0 + 1.0
            ),
        )
        opcode = dve_ops._CUSTOM_DVE_ROW_BASE + len(dve_ops.OPS)
        shas = {}
        for ver in ("v3", "v4"):
            sp = DveOpSpec(
                name=name, opcode=opcode, uops=lower(spec, ver=ver), rd1_en=False
            )
            shas[ver] = sp.sha(ver)
        op = dve_ops.DveOp(name, spec, subdim=False, uops_sha=shas)
        dve_ops.OPS.append(op)
        dve_ops._SUB_OPCODE_FOR_NAME[name] = opcode
        dve_ops.CUSTOM_DVE_SPECS[name] = spec
    _EXP3 = next(o for o in dve_ops.OPS if o.name == name)
    return _EXP3


def build_program(repeat=1, bench_io=False):
    import concourse.bacc as bacc
    import concourse.mybir as mybir
    import concourse.tile as tile
    from contextlib import ExitStack

    exp3 = _register_exp3()

    dt = mybir.dt
    f32, bf16, fp8 = dt.float32, dt.bfloat16, dt.float8e4
    AX = mybir.AxisListType
    AF = mybir.ActivationFunctionType
    DR = mybir.MatmulPerfMode.DoubleRow

    nc = bacc.Bacc("TRN2", target_bir_lowering=False, debug=False)

    io_kind_in = "Internal" if bench_io else "ExternalInput"
    io_kind_out = "Internal" if bench_io else "ExternalOutput"
    x_d = nc.dram_tensor("x", [BPC, N, CH], f32, kind=io_kind_in).ap()
    wqkv8_d = nc.dram_tensor(
        "wqkv8", [2, 128, 2, 3 * HIDDEN], fp8, kind="ExternalInput"
    ).ap()
    wout_d = nc.dram_tensor("wout", [HIDDEN, CH], bf16, kind="ExternalInput").ap()
    identf_d = nc.dram_tensor("identf", [128, 128], f32, kind="ExternalInput").ap()
    identb_d = nc.dram_tensor("identb", [128, 128], bf16, kind="ExternalInput").ap()
    sel32_d = nc.dram_tensor("sel32", [32, 128], f32, kind="ExternalInput").ap()
    mask32_d = nc.dram_tensor("mask32", [32, 4], f32, kind="ExternalInput").ap()
    gns_d = nc.dram_tensor("gns", [128, 4], f32, kind="ExternalInput").ap()
    gno_d = nc.dram_tensor("gno", [128, 4], f32, kind="ExternalInput").ap()
    bb_d = nc.dram_tensor("bb", [128, CH], f32, kind="ExternalInput").ap()
    ones_d = nc.dram_tensor("ones", [128, 1], f32, kind="ExternalInput").ap()
    out_d = nc.dram_tensor("out", [BPC, N, CH], f32, kind=io_kind_out).ap()
    tout_d = (
        nc.dram_tensor("tout", [128, 16], f32, kind="ExternalOutput").ap()
        if bench_io
        else None
    )

    with ExitStack() as ctx:
        tc = ctx.enter_context(tile.TileContext(nc))
        pc = ctx.enter_context(tc.tile_pool(name="const", bufs=1))
        px = ctx.enter_context(tc.tile_pool(name="px", bufs=3))
        psq = ctx.enter_context(tc.tile_pool(name="psq", bufs=2))
        pst = ctx.enter_context(tc.tile_pool(name="pst", bufs=4))
        ptiny = ctx.enter_context(tc.tile_pool(name="ptiny", bufs=2))
        pxnT = ctx.enter_context(tc.tile_pool(name="pxnT", bufs=4))
        pq = ctx.enter_context(tc.tile_pool(name="pq", bufs=8))
        pk = ctx.enter_context(tc.tile_pool(name="pk", bufs=8))
        pv = ctx.enter_context(tc.tile_pool(name="pv", bufs=16))
        pe = ctx.enter_context(tc.tile_pool(name="pe", bufs=14))
        pao = ctx.enter_context(tc.tile_pool(name="pao", bufs=3))
        paoT = ctx.enter_context(tc.tile_pool(name="paoT", bufs=8))
        prc = ctx.enter_context(tc.tile_pool(name="prc", bufs=4))
        pout = ctx.enter_context(tc.tile_pool(name="pout", bufs=2))
        pps = ctx.enter_context(tc.tile_pool(name="pps", bufs=2, space="PSUM"))
        ppsim = ctx.enter_context(tc.tile_pool(name="ppsim", bufs=2, space="PSUM"))
        pppv = ctx.enter_context(tc.tile_pool(name="pppv", bufs=2, space="PSUM"))

        state = {}

        def emit_xload(bi, b):
            s = {}
            xb = px.tile([128, NT * CH], f32, name=f"xb{bi}", tag="x")
            for c4 in range(4):
                nc.sync.dma_start(
                    out=xb[:, 2 * CH * c4 : 2 * CH * (c4 + 1)].rearrange(
                        "p (t c) -> p t c", t=2
                    ),
                    in_=x_d[b, 256 * c4 : 256 * (c4 + 1), :].rearrange(
                        "(t p) c -> p t c", p=128
                    ),
                )
            s["xb"] = xb
            state[bi] = s

        emit_xload(0, 0)

        # ---- constants ----
        wqkv8 = []
        for j in range(2):
            t = pc.tile([128, 2, 3 * HIDDEN], fp8, name=f"wqkv8{j}", tag=f"wqkv8{j}")
            nc.sync.dma_start(out=t[:], in_=wqkv8_d[j])
            wqkv8.append(t)
        wout = []
        for j in range(CC):
            t = pc.tile([128, CH], bf16, name=f"wout{j}", tag=f"wout{j}")
            nc.sync.dma_start(out=t[:], in_=wout_d[128 * j : 128 * (j + 1), :])
            wout.append(t)
        identf = pc.tile([128, 128], f32, name="identf", tag="identf")
        nc.sync.dma_start(out=identf[:], in_=identf_d[:, :])
        identb = pc.tile([128, 128], bf16, name="identb", tag="identb")
        nc.sync.dma_start(out=identb[:], in_=identb_d[:, :])
        sel32 = pc.tile([32, 128], f32, name="sel32", tag="sel32")
        nc.sync.dma_start(out=sel32[:], in_=sel32_d[:, :])
        mask32 = pc.tile([32, 4], f32, name="mask32", tag="mask32")
        nc.sync.dma_start(out=mask32[:], in_=mask32_d[:, :])
        gns = pc.tile([128, 4], f32, name="gns", tag="gns")
        nc.sync.dma_start(out=gns[:], in_=gns_d[:, :])
        gno = pc.tile([128, 4], f32, name="gno", tag="gno")
        nc.sync.dma_start(out=gno[:], in_=gno_d[:, :])
        bb = pc.tile([128, CH], f32, name="bb", tag="bb")
        nc.sync.dma_start(out=bb[:], in_=bb_d[:, :])
        ones = pc.tile([128, 1], f32, name="ones", tag="ones")
        nc.sync.dma_start(out=ones[:], in_=ones_d[:, :])

        def make_prologue_chunks(bi, b):
            s = state[bi]
            xb = s["xb"]

            def emit_all():

                # GroupNorm stats
                ps_st = pppv.tile([32, 2], f32, name=f"ps_st{bi}", tag="pv")
                for nt in range(NT):
                    st = pst.tile([128, 64], f32, name=f"st{bi}_{nt}", tag="stats")
                    xv = xb[:, CH * nt : CH * (nt + 1)].rearrange(
                        "p (g k) -> p g k", g=GROUPS
                    )
                    nc.vector.reduce_sum(out=st[:, 0:32], in_=xv, axis=AX.X)
                    sq = psq.tile([128, CH], f32, name=f"sq{bi}_{nt}", tag="sq")
                    nc.gpsimd.tensor_mul(
                        sq[:], xb[:, CH * nt : CH * (nt + 1)], xb[:, CH * nt : CH * (nt + 1)]
                    )
                    nc.vector.reduce_sum(
                        out=st[:, 32:64],
                        in_=sq[:].rearrange("p (g k) -> p g k", g=GROUPS),
                        axis=AX.X,
                    )
                    nc.tensor.matmul(
                        out=ps_st[:, 0:1], lhsT=st[:, 0:32], rhs=ones[:],
                        start=(nt == 0), stop=False,
                    )
                    nc.tensor.matmul(
                        out=ps_st[:, 1:2], lhsT=st[:, 32:64], rhs=ones[:],
                        start=False, stop=(nt == NT - 1),
                    )
                    yield

                yield
                # group mean/rstd -> per-channel affine A, B [128, 4]
                g1 = ptiny.tile([32, 8], f32, name=f"g1{bi}", tag="g1")
                inv_n = 1.0 / (N * (CH // GROUPS))
                nc.vector.tensor_scalar_mul(g1[:, 0:1], ps_st[:, 0:1], inv_n)  # mean
                nc.vector.tensor_scalar_mul(g1[:, 1:2], ps_st[:, 1:2], inv_n)  # E[x^2]
                nc.vector.tensor_mul(g1[:, 2:3], g1[:, 0:1], g1[:, 0:1])
                nc.vector.tensor_sub(g1[:, 3:4], g1[:, 1:2], g1[:, 2:3])  # var
                nc.vector.tensor_scalar_add(g1[:, 4:5], g1[:, 3:4], EPS)
                # rstd = 1/sqrt(w) via Newton on DVE (w = var+eps ~ 1, so
                # y0 = 1.5 - w/2 then two y *= 1.5 - w*y^2/2 steps converge
                # to fp32) -- avoids the AF.Sqrt activation-table swap
                w_ = g1[:, 4:5]
                y_ = g1[:, 5:6]
                t_ = g1[:, 6:7]
                nc.vector.tensor_scalar(
                    out=y_, in0=w_, scalar1=-0.5, scalar2=1.5,
                    op0=mybir.AluOpType.mult, op1=mybir.AluOpType.add,
                )
                for _ in range(2):
                    nc.vector.tensor_mul(t_, y_, y_)
                    nc.vector.tensor_mul(t_, t_, w_)
                    nc.vector.tensor_scalar(
                        out=t_, in0=t_, scalar1=-0.5, scalar2=1.5,
                        op0=mybir.AluOpType.mult, op1=mybir.AluOpType.add,
                    )
                    nc.vector.tensor_mul(y_, y_, t_)
                selr = ptiny.tile([32, 8], f32, name=f"selr{bi}", tag="selr")
                nc.vector.tensor_scalar_mul(selr[:, 0:4], mask32[:], g1[:, 5:6])
                nc.vector.tensor_scalar_mul(selr[:, 4:8], mask32[:], g1[:, 0:1])
                ps_ab = pppv.tile([128, 8], f32, name=f"ps_ab{bi}", tag="pv")
                nc.tensor.matmul(out=ps_ab[:], lhsT=sel32[:], rhs=selr[:])
                A = ptiny.tile([128, 4], f32, name=f"A{bi}", tag="A")
                Bt = ptiny.tile([128, 4], f32, name=f"Bt{bi}", tag="Bt")
                tmb = ptiny.tile([128, 4], f32, name=f"tmb{bi}", tag="tmb")
                nc.vector.tensor_mul(A[:], ps_ab[:, 0:4], gns[:])
                nc.vector.tensor_mul(tmb[:], ps_ab[:, 4:8], A[:])
                nc.vector.tensor_sub(Bt[:], gno[:], tmb[:])
                An = ptiny.tile([128, 4], f32, name=f"An{bi}", tag="An")
                Bn = ptiny.tile([128, 4], f32, name=f"Bn{bi}", tag="Bn")
                nc.vector.tensor_scalar_mul(An[:], A[:], -1.0)
                nc.vector.tensor_scalar_mul(Bn[:], Bt[:], -1.0)

                yield
                # transposed normalize: silu(x^T * A + B) -> fp8 xnT packed
                # [128, 2(c-sub), 1024] per 256-channel pair for DoubleRow;
                # channel c = 128*(2j + jj) + p lives at xnT[j][p, jj, :]
                xnT = [
                    pxnT.tile([128, 2, N], fp8, name=f"xnT{bi}_{j}", tag="xnT")
                    for j in range(2)
                ]
                for jc in range(CC):
                    for half in range(2):
                        pt = pps.tile(
                            [128, 512], f32, name=f"pt{bi}_{jc}_{half}", tag="ps512"
                        )
                        for q in range(4):
                            nt = 4 * half + q
                            nc.tensor.matmul(
                                out=pt[:, 128 * q : 128 * (q + 1)],
                                lhsT=xb[:, CH * nt + 128 * jc : CH * nt + 128 * (jc + 1)],
                                rhs=identf[:],
                                is_transpose=True,
                                start=(q == 0), stop=(q == 3),
                            )
                        u = ptiny.tile([128, 512], f32, name=f"u{bi}_{jc}_{half}", tag="u")
                        nc.vector.tensor_scalar(
                            out=u[:], in0=pt[:],
                            scalar1=A[:, jc : jc + 1], scalar2=Bt[:, jc : jc + 1],
                            op0=mybir.AluOpType.mult, op1=mybir.AluOpType.add,
                        )
                        # silu(u) = u / (1 + exp(-u))
                        sg = ptiny.tile(
                            [128, 512], f32, name=f"sg{bi}_{jc}_{half}", tag="sg"
                        )
                        nc.scalar.activation(
                            sg[:], pt[:], AF.Exp,
                            bias=Bn[:, jc : jc + 1], scale=An[:, jc : jc + 1],
                        )
                        nc.vector.tensor_scalar_add(sg[:], sg[:], 1.0)
                        nc.vector.reciprocal(sg[:], sg[:])
                        nc.gpsimd.tensor_mul(
                            xnT[jc // 2][:, jc % 2, 512 * half : 512 * (half + 1)],
                            u[:], sg[:],
                        )
                        yield

                yield
                # QKV projections (fp8 DoubleRow, contraction 512 = 2x256):
                # q, k -> [d, n] fp8; v -> [n, 8*65] fp8 with ones columns
                qt = [pq.tile([128, N], fp8, name=f"q{bi}_{dc}", tag="q") for dc in range(CC)]
                kt = [pk.tile([128, N], fp8, name=f"k{bi}_{dc}", tag="k") for dc in range(CC)]
                for which, dst in ((0, qt), (1, kt)):
                    if which == 1:
                        yield
                    for dc in range(CC):
                        for half in range(2):
                            pp = pps.tile(
                                [128, 512], f32, name=f"pqk{bi}_{which}_{dc}_{half}",
                                tag="ps512",
                            )
                            cb = 512 * which + 128 * dc
                            for j in range(2):
                                nc.tensor.matmul(
                                    out=pp[:],
                                    lhsT=wqkv8[j][:, :, cb : cb + 128],
                                    rhs=xnT[j][:, :, 512 * half : 512 * (half + 1)],
                                    perf_mode=DR,
                                    start=(j == 0), stop=(j == 1),
                                )
                            if which == 0:
                                nc.scalar.activation(
                                    dst[dc][:, 512 * half : 512 * (half + 1)], pp[:], AF.Copy
                                )
                            else:
                                nc.vector.tensor_copy(
                                    dst[dc][:, 512 * half : 512 * (half + 1)], pp[:]
                                )
                        yield
                yield
                vt = []
                for nt in range(NT):
                    t = pv.tile([128, HEADS * 65], fp8, name=f"v{bi}_{nt}", tag="v")
                    vt.append(t)
                    nc.gpsimd.memset(
                        t[:].rearrange("p (h x) -> p h x", h=HEADS)[:, :, 64:65], 1.0
                    )
                    pp = pps.tile([128, 512], f32, name=f"pv{bi}_{nt}", tag="ps512")
                    for j in range(2):
                        nc.tensor.matmul(
                            out=pp[:],
                            lhsT=xnT[j][:, :, 128 * nt : 128 * (nt + 1)],
                            rhs=wqkv8[j][:, :, 1024:1536],
                            perf_mode=DR,
                            start=(j == 0), stop=(j == 1),
                        )
                    nc.scalar.activation(
                        t[:].rearrange("p (h x) -> p h x", h=HEADS)[:, :, 0:64],
                        pp[:].rearrange("p (h x) -> p h x", h=HEADS),
                        AF.Copy,
                    )
                    if nt % 2 == 1:
                        yield
                yield
                s["qt"], s["kt"], s["vt"] = qt, kt, vt

            gen = emit_all()

            def pull():
                try:
                    next(gen)
                except StopIteration:
                    pass

            return [pull] * 40

        def attention(bi, extra=None):
            s = state[bi]
            qt, kt, vt = s["qt"], s["kt"], s["vt"]
            ao = pao.tile([128, NT * HIDDEN], bf16, name=f"ao{bi}", tag="ao")
            # DVE share of exp tiles per head (cycle of 4 heads)
            DVE_PATTERNS = ((2, 5), (2, 5), (1, 4, 6), (2, 5, 7))

            def emit_sim_exp(h, jt):
                dc = h // 2
                r0 = 64 * (h % 2)
                psim = ppsim.tile([128, N], f32, name=f"psim{bi}_{h}_{jt}", tag="sim")
                for half in range(2):
                    nc.tensor.matmul(
                        out=psim[:, 512 * half : 512 * (half + 1)],
                        lhsT=kt[dc][r0 : r0 + 64, 128 * jt : 128 * (jt + 1)],
                        rhs=qt[dc][r0 : r0 + 64, 512 * half : 512 * (half + 1)],
                    )
                et = pe.tile([128, N], fp8, name=f"eT{bi}_{h}_{jt}", tag="eT")
                if jt in DVE_PATTERNS[h % 4]:
                    nc.vector._custom_dve(
                        exp3, out=et[:], in0=psim[:],
                        s0=EXP_SCALE**3 / 6.0, s1=EXP_SCALE**2 / 2.0, imm2=EXP_SCALE,
                    )
                else:
                    nc.scalar.activation(et[:], psim[:], AF.Exp, scale=EXP_SCALE)
                return et

            def new_pvctx(h, eT):
                ppvs = [
                    pppv.tile([128, 4 * 65], f32, name=f"ppv{bi}_{h}_{ig}", tag="pv")
                    for ig in range(2)
                ]
                return (h, eT, ppvs)

            def emit_pv_chunk(ctx_pv, jt):
                h, eT, ppvs = ctx_pv
                for ig in range(2):
                    for ii in range(4):
                        it = 4 * ig + ii
                        nc.tensor.matmul(
                            out=ppvs[ig][:, 65 * ii : 65 * (ii + 1)],
                            lhsT=eT[jt][:, 128 * it : 128 * (it + 1)],
                            rhs=vt[jt][:, 65 * h : 65 * (h + 1)],
                            start=(jt == 0 and ii == 0),
                            stop=(jt == NT - 1 and ii == 3),
                        )

            def emit_pv_drain(ctx_pv):
                h, eT, ppvs = ctx_pv
                for ig in range(2):
                    ppv = ppvs[ig]
                    rc4 = prc.tile([128, 4], f32, name=f"rc4{bi}_{h}_{ig}", tag="rc")
                    ppv_v = ppv[:].rearrange("p (i x) -> p i x", x=65)
                    nc.vector.reciprocal(rc4[:], ppv_v[:, :, 64:65])
                    nc.vector.tensor_mul(
                        ao[:].rearrange("p (i c) -> p i c", i=NT)[
                            :, 4 * ig : 4 * ig + 4, 64 * h : 64 * (h + 1)
                        ],
                        ppv_v[:, :, 0:64],
                        rc4[:].rearrange("p (i o) -> p i o", o=1).broadcast_to(
                            [128, 4, 64]
                        ),
                    )

            # 1-head software pipeline, interleaved at j-tile granularity
            pvctx = None
            for h in range(HEADS):
                for f in (extra[h::HEADS] if extra else ()):
                    f()
                eT = []
                for jt in range(NT):
                    eT.append(emit_sim_exp(h, jt))
                    if pvctx is not None:
                        emit_pv_chunk(pvctx, jt)
                if pvctx is not None:
                    emit_pv_drain(pvctx)
                pvctx = new_pvctx(h, eT)
            for jt in range(NT):
                emit_pv_chunk(pvctx, jt)
            emit_pv_drain(pvctx)
            s["ao"] = ao

        def make_epilogue_chunks(bi, b):
            s = state[bi]
            xb, ao = s["xb"], s["ao"]
            chunks = []
            aoT = [
                paoT.tile([128, N], bf16, name=f"aoT{bi}_{dc}", tag="aoT")
                for dc in range(CC)
            ]

            def aot_chunk(dc2):
                for half in range(2):
                    pt2 = pps.tile(
                        [128, 512], bf16, name=f"pt2{bi}_{dc2}_{half}", tag="ps512"
                    )
                    for q in range(4):
                        nt = 4 * half + q
                        nc.tensor.matmul(
                            out=pt2[:, 128 * q : 128 * (q + 1)],
                            lhsT=ao[
                                :, HIDDEN * nt + 128 * dc2 : HIDDEN * nt + 128 * (dc2 + 1)
                            ],
                            rhs=identb[:],
                            is_transpose=True,
                            start=(q == 0), stop=(q == 3),
                        )
                    nc.scalar.activation(
                        aoT[dc2][:, 512 * half : 512 * (half + 1)], pt2[:], AF.Copy
                    )

            for dc2 in range(CC):
                chunks.append(lambda dc2=dc2: aot_chunk(dc2))
            ob = pout.tile([128, NT * CH], f32, name=f"ob{bi}", tag="ob")

            def oproj_chunk(g):
                for nt in (2 * g, 2 * g + 1):
                    pf = pps.tile([128, CH], f32, name=f"pf{bi}_{nt}", tag="ps512")
                    for dc2 in range(CC):
                        nc.tensor.matmul(
                            out=pf[:],
                            lhsT=aoT[dc2][:, 128 * nt : 128 * (nt + 1)],
                            rhs=wout[dc2][:],
                            start=(dc2 == 0), stop=(dc2 == CC - 1),
                        )
                    nc.vector.tensor_add(
                        ob[:, CH * nt : CH * (nt + 1)], pf[:],
                        xb[:, CH * nt : CH * (nt + 1)],
                    )
                    nc.gpsimd.tensor_add(
                        ob[:, CH * nt : CH * (nt + 1)],
                        ob[:, CH * nt : CH * (nt + 1)], bb[:],
                    )
                nc.sync.dma_start(
                    out=out_d[b, 256 * g : 256 * (g + 1), :].rearrange(
                        "(t p) c -> p t c", p=128
                    ),
                    in_=ob[:, 2 * CH * g : 2 * CH * (g + 1)].rearrange(
                        "p (t c) -> p t c", t=2
                    ),
                )

            for g in range(4):
                chunks.append(lambda g=g: oproj_chunk(g))
            return chunks

        # cross-group software pipeline: batch b's prologue and the previous
        # batch's epilogue interleave into the current attention, so in a
        # repeat loop (and in the 4-batch steady state generally) no engine
        # drains between groups
        epi_prev = None
        for g in range(repeat):
            b0, b1 = 2 * g, 2 * g + 1
            if g == 0:
                for f in make_prologue_chunks(b0, 0):
                    f()
            emit_xload(b1, 1)
            extra0 = make_prologue_chunks(b1, 1) + (epi_prev or [])
            attention(b0, extra=extra0)
            extra1 = make_epilogue_chunks(b0, 0)
            del state[b0]
            if g + 1 < repeat:
                emit_xload(b0 + 2, 0)
                extra1 = extra1 + make_prologue_chunks(b0 + 2, 0)
            attention(b1, extra=extra1)
            epi_prev = make_epilogue_chunks(b1, 1)
            del state[b1]
        for f in epi_prev:
            f()
        if tout_d is not None:
            tt = pc.tile([128, 16], f32, name="tt", tag="tt")
            nc.vector.memset(tt[:], 1.0)
            nc.sync.dma_start(out=tout_d[:, :], in_=tt[:])

    nc.compile()
    return nc


def make_in_maps(x, gn_scale, gn_offset, w_qkv, w_out, b_out):
    import ml_dtypes

    bf16 = ml_dtypes.bfloat16
    fp8 = ml_dtypes.float8_e4m3
    x = np.asarray(x, dtype=np.float32)
    gn_scale = np.asarray(gn_scale, dtype=np.float32)
    gn_offset = np.asarray(gn_offset, dtype=np.float32)
    w_qkv = np.asarray(w_qkv, dtype=np.float32)
    w_out = np.asarray(w_out, dtype=np.float32)
    b_out = np.asarray(b_out, dtype=np.float32)

    # 16x scaling into fp8; channel c = 128*(2j+jj)+p -> wqkv8[j, p, jj, :]
    w16 = (W_SCALE * w_qkv).reshape(2, 2, 128, 3 * HIDDEN).transpose(0, 2, 1, 3)
    wqkv8 = np.ascontiguousarray(w16.astype(fp8))

    wout_h = np.ascontiguousarray((w_out / W_SCALE).astype(bf16))
    identf = np.eye(128, dtype=np.float32)
    identb = np.eye(128, dtype=np.float32).astype(bf16)
    g_idx = np.arange(32)
    sel32 = (g_idx[:, None] % 8 == np.arange(128)[None, :] // 16).astype(np.float32)
    mask32 = (g_idx[:, None] // 8 == np.arange(4)[None, :]).astype(np.float32)
    gns = np.ascontiguousarray(gn_scale.reshape(4, 128).T.astype(np.float32))
    gno = np.ascontiguousarray(gn_offset.reshape(4, 128).T.astype(np.float32))
    bb = np.broadcast_to(b_out, (128, CH)).copy()
    ones = np.ones((128, 1), dtype=np.float32)

    xr = x.reshape(B, N, CH)
    in_maps = []
    for i in range(N_CORES):
        in_maps.append(
            {
                "x": np.ascontiguousarray(xr[BPC * i : BPC * (i + 1)]),
                "wqkv8": wqkv8,
                "wout": wout_h,
                "identf": identf,
                "identb": identb,
                "sel32": sel32,
                "mask32": mask32,
                "gns": gns,
                "gno": gno,
                "bb": bb,
                "ones": ones,
            }
        )
    return in_maps


_NC_CACHE = None


def kernel(x, gn_scale, gn_offset, w_qkv, w_out, b_out, _return_extra=False):
    global _NC_CACHE
    from concourse.bass_utils import run_bass_kernel_spmd

    if _NC_CACHE is None:
        _NC_CACHE = build_program()
    nc = _NC_CACHE
    in_maps = make_in_maps(x, gn_scale, gn_offset, w_qkv, w_out, b_out)
    res = run_bass_kernel_spmd(nc, in_maps, list(range(N_CORES)))
    outs = [res.results[i]["out"] for i in range(N_CORES)]
    out = np.concatenate(outs, axis=0).reshape(B, HGT, WID, CH).astype(np.float32)
    if _return_extra:
        return out, res
    return out


# revision 8
# speedup vs baseline: 2.4394x; 2.4394x over previous
"""Trainium2 Bass kernel for nn_Attention_36146444763783.

GroupNorm(32) + SiLU -> QKV proj -> 8-head attention (n=1024) -> out proj
+ bias + residual, batch=16, fully data-parallel: 2 batches per NeuronCore
across 8 cores.

Per-core dataflow (fp8 DoubleRow matmuls for the QKV projections, plain fp8
for logits + PV, bf16 for out proj; fp32 PSUM everywhere):
  - x [2,1024,512] fp32 loaded as [128, 8*512] tiles (partition = token%128)
  - GroupNorm stats per (batch, group) via DVE reduces + PE
    ones-matmul partition sums; per-channel affine A,B expanded to [128,4]
    via a selector matmul; normalize+SiLU runs on PE-transposed x blocks,
    final mul writes fp8 xnT packed [128, 2(c-sub), 1024] so the QKV
    matmuls can run DoubleRow (contraction 512 done as 2x256)
  - wqkv host-scaled 16x into fp8 (so q,k,v = 16*raw); q,k stored [d, n]
    fp8, v stored [n, 8*65] fp8 with a sumexp ones column per head
  - logits: simT[j,i] = k^T q, plain fp8 matmuls (contraction 64);
    exp(logit) = exp(psum * 2^-11) via ScalarE activation-scale and a
    custom degree-3 DVE poly op, split by a static pattern; fp8 eT out
  - PV: plain fp8 matmuls (FD=65: attn-out + sumexp column), fp8 FWL keeps
    weight loads hidden; PSUM drained with a broadcast normalize
    (reciprocal of the sumexp column) into bf16 ao
  - out proj bf16 from PE-transposed ao with wout/16 host-folded; residual
    and bias adds on DVE (GPSIMD measured ~2x slower per op on HW - unused)
  - both batches' prologues are emitted before attention so the second
    batch's GroupNorm/QKV overlaps the first batch's attention
"""

import sys

import numpy as np

sys.path.insert(0, "/opt/trn_rl_repo")

B, HGT, WID, CH = 16, 32, 32, 512
HEADS, HEAD_CH, HIDDEN = 8, 64, 512
GROUPS = 32
EPS = 1e-5
N = HGT * WID  # 1024 tokens per batch
N_CORES = 8
BPC = B // N_CORES  # batches per core
NT = N // 128  # 8 token tiles
CC = CH // 128  # 4 channel chunks

W_SCALE = 16.0  # fp8 range scaling for wqkv (and q,k,v downstream)
EXP_SCALE = 1.0 / (W_SCALE * W_SCALE * (HEAD_CH**0.5))  # 2^-11

_EXP3 = None


def _register_exp3():
    """Degree-3 polynomial for exp(s*x) as a custom DVE op:
    1 + (sx) + (sx)^2/2 + (sx)^3/6 with the scale baked into the
    coefficients. Valid for |sx| <= ~0.5."""
    global _EXP3
    if _EXP3 is not None:
        return _EXP3
    from concourse import dve_ops
    from concourse.dve_spec import Spec, Src0, C0, C1, C2, One, lower
    from concourse.dve_uop import DveOpSpec

    name = "EXP3_ANT"
    if name not in dve_ops._SUB_OPCODE_FOR_NAME:
        body = ((Src0 * C0 + C1) * Src0 + C2) * Src0 + One
        spec = Spec(
            body=body,
            reference=lambda in0, in1, s0, s1, imm2: (
                ((in0 * s0 + s1) * in0 + imm2) * in0 + 1.0
            ),
        )
        opcode = dve_ops._CUSTOM_DVE_ROW_BASE + len(dve_ops.OPS)
        shas = {}
        for ver in ("v3", "v4"):
            sp = DveOpSpec(
                name=name, opcode=opcode, uops=lower(spec, ver=ver), rd1_en=False
            )
            shas[ver] = sp.sha(ver)
        op = dve_ops.DveOp(name, spec, subdim=False, uops_sha=shas)
        dve_ops.OPS.append(op)
        dve_ops._SUB_OPCODE_FOR_NAME[name] = opcode
        dve_ops.CUSTOM_DVE_SPECS[name] = spec
    _EXP3 = next(o for o in dve_ops.OPS if o.name == name)
    return _EXP3


def build_program(repeat=1, bench_io=False):
    import concourse.bacc as bacc
    import concourse.mybir as mybir
    import concourse.tile as tile
    from contextlib import ExitStack

    exp3 = _register_exp3()

    dt = mybir.dt
    f32, bf16, fp8 = dt.float32, dt.bfloat16, dt.float8e4
    AX = mybir.AxisListType
    AF = mybir.ActivationFunctionType
    DR = mybir.MatmulPerfMode.DoubleRow

    nc = bacc.Bacc("TRN2", target_bir_lowering=False, debug=False)

    io_kind_in = "Internal" if bench_io else "ExternalInput"
    io_kind_out = "Internal" if bench_io else "ExternalOutput"
    x_d = nc.dram_tensor("x", [BPC, N, CH], f32, kind=io_kind_in).ap()
    wqkv8_d = nc.dram_tensor(
        "wqkv8", [2, 128, 2, 3 * HIDDEN], fp8, kind="ExternalInput"
    ).ap()
    wout_d = nc.dram_tensor("wout", [HIDDEN, CH], bf16, kind="ExternalInput").ap()
    identf_d = nc.dram_tensor("identf", [128, 128], f32, kind="ExternalInput").ap()
    identb_d = nc.dram_tensor("identb", [128, 128], bf16, kind="ExternalInput").ap()
    sel32_d = nc.dram_tensor("sel32", [32, 128], f32, kind="ExternalInput").ap()
    mask32_d = nc.dram_tensor("mask32", [32, 4], f32, kind="ExternalInput").ap()
    gns_d = nc.dram_tensor("gns", [128, 4], f32, kind="ExternalInput").ap()
    gno_d = nc.dram_tensor("gno", [128, 4], f32, kind="ExternalInput").ap()
    bb_d = nc.dram_tensor("bb", [128, CH], f32, kind="ExternalInput").ap()
    ones_d = nc.dram_tensor("ones", [128, 1], f32, kind="ExternalInput").ap()
    out_d = nc.dram_tensor("out", [BPC, N, CH], f32, kind=io_kind_out).ap()
    tout_d = (
        nc.dram_tensor("tout", [128, 16], f32, kind="ExternalOutput").ap()
        if bench_io
        else None
    )

    with ExitStack() as ctx:
        tc = ctx.enter_context(tile.TileContext(nc))
        pc = ctx.enter_context(tc.tile_pool(name="const", bufs=1))
        px = ctx.enter_context(tc.tile_pool(name="px", bufs=3))
        psq = ctx.enter_context(tc.tile_pool(name="psq", bufs=2))
        pst = ctx.enter_context(tc.tile_pool(name="pst", bufs=4))
        ptiny = ctx.enter_context(tc.tile_pool(name="ptiny", bufs=2))
        pxnT = ctx.enter_context(tc.tile_pool(name="pxnT", bufs=4))
        pq = ctx.enter_context(tc.tile_pool(name="pq", bufs=8))
        pk = ctx.enter_context(tc.tile_pool(name="pk", bufs=8))
        pv = ctx.enter_context(tc.tile_pool(name="pv", bufs=16))
        pe = ctx.enter_context(tc.tile_pool(name="pe", bufs=14))
        pao = ctx.enter_context(tc.tile_pool(name="pao", bufs=3))
        paoT = ctx.enter_context(tc.tile_pool(name="paoT", bufs=8))
        prc = ctx.enter_context(tc.tile_pool(name="prc", bufs=4))
        pout = ctx.enter_context(tc.tile_pool(name="pout", bufs=2))
        pps = ctx.enter_context(tc.tile_pool(name="pps", bufs=2, space="PSUM"))
        ppsim = ctx.enter_context(tc.tile_pool(name="ppsim", bufs=2, space="PSUM"))
        pppv = ctx.enter_context(tc.tile_pool(name="pppv", bufs=2, space="PSUM"))

        state = {}

        def emit_xload(bi, b):
            s = {}
            xb = px.tile([128, NT * CH], f32, name=f"xb{bi}", tag="x")
            for c4 in range(4):
                nc.sync.dma_start(
                    out=xb[:, 2 * CH * c4 : 2 * CH * (c4 + 1)].rearrange(
                        "p (t c) -> p t c", t=2
                    ),
                    in_=x_d[b, 256 * c4 : 256 * (c4 + 1), :].rearrange(
                        "(t p) c -> p t c", p=128
                    ),
                )
            s["xb"] = xb
            state[bi] = s

        emit_xload(0, 0)

        # ---- constants ----
        wqkv8 = []
        for j in range(2):
            t = pc.tile([128, 2, 3 * HIDDEN], fp8, name=f"wqkv8{j}", tag=f"wqkv8{j}")
            nc.sync.dma_start(out=t[:], in_=wqkv8_d[j])
            wqkv8.append(t)
        wout = []
        for j in range(CC):
            t = pc.tile([128, CH], bf16, name=f"wout{j}", tag=f"wout{j}")
            nc.sync.dma_start(out=t[:], in_=wout_d[128 * j : 128 * (j + 1), :])
            wout.append(t)
        identf = pc.tile([128, 128], f32, name="identf", tag="identf")
        nc.sync.dma_start(out=identf[:], in_=identf_d[:, :])
        identb = pc.tile([128, 128], bf16, name="identb", tag="identb")
        nc.sync.dma_start(out=identb[:], in_=identb_d[:, :])
        sel32 = pc.tile([32, 128], f32, name="sel32", tag="sel32")
        nc.sync.dma_start(out=sel32[:], in_=sel32_d[:, :])
        mask32 = pc.tile([32, 4], f32, name="mask32", tag="mask32")
        nc.sync.dma_start(out=mask32[:], in_=mask32_d[:, :])
        gns = pc.tile([128, 4], f32, name="gns", tag="gns")
        nc.sync.dma_start(out=gns[:], in_=gns_d[:, :])
        gno = pc.tile([128, 4], f32, name="gno", tag="gno")
        nc.sync.dma_start(out=gno[:], in_=gno_d[:, :])
        bb = pc.tile([128, CH], f32, name="bb", tag="bb")
        nc.sync.dma_start(out=bb[:], in_=bb_d[:, :])
        ones = pc.tile([128, 1], f32, name="ones", tag="ones")
        nc.sync.dma_start(out=ones[:], in_=ones_d[:, :])

        def make_prologue_chunks(bi, b):
            s = state[bi]
            xb = s["xb"]

            def emit_all():

                # GroupNorm stats
                ps_st = pppv.tile([32, 2], f32, name=f"ps_st{bi}", tag="pv")
                for nt in range(NT):
                    st = pst.tile([128, 64], f32, name=f"st{bi}_{nt}", tag="stats")
                    xv = xb[:, CH * nt : CH * (nt + 1)].rearrange(
                        "p (g k) -> p g k", g=GROUPS
                    )
                    nc.vector.reduce_sum(out=st[:, 0:32], in_=xv, axis=AX.X)
                    sq = psq.tile([128, CH], f32, name=f"sq{bi}_{nt}", tag="sq")
                    nc.vector.tensor_mul(
                        sq[:], xb[:, CH * nt : CH * (nt + 1)], xb[:, CH * nt : CH * (nt + 1)]
                    )
                    nc.vector.reduce_sum(
                        out=st[:, 32:64],
                        in_=sq[:].rearrange("p (g k) -> p g k", g=GROUPS),
                        axis=AX.X,
                    )
                    nc.tensor.matmul(
                        out=ps_st[:, 0:1], lhsT=st[:, 0:32], rhs=ones[:],
                        start=(nt == 0), stop=False,
                    )
                    nc.tensor.matmul(
                        out=ps_st[:, 1:2], lhsT=st[:, 32:64], rhs=ones[:],
                        start=False, stop=(nt == NT - 1),
                    )
                    yield

                yield
                # group mean/rstd -> per-channel affine A, B [128, 4]
                g1 = ptiny.tile([32, 8], f32, name=f"g1{bi}", tag="g1")
                inv_n = 1.0 / (N * (CH // GROUPS))
                nc.vector.tensor_scalar_mul(g1[:, 0:1], ps_st[:, 0:1], inv_n)  # mean
                nc.vector.tensor_scalar_mul(g1[:, 1:2], ps_st[:, 1:2], inv_n)  # E[x^2]
                nc.vector.tensor_mul(g1[:, 2:3], g1[:, 0:1], g1[:, 0:1])
                nc.vector.tensor_sub(g1[:, 3:4], g1[:, 1:2], g1[:, 2:3])  # var
                nc.vector.tensor_scalar_add(g1[:, 4:5], g1[:, 3:4], EPS)
                # rstd = 1/sqrt(w) via Newton on DVE (w = var+eps ~ 1, so
                # y0 = 1.5 - w/2 then two y *= 1.5 - w*y^2/2 steps converge
                # to fp32) -- avoids the AF.Sqrt activation-table swap
                w_ = g1[:, 4:5]
                y_ = g1[:, 5:6]
                t_ = g1[:, 6:7]
                nc.vector.tensor_scalar(
                    out=y_, in0=w_, scalar1=-0.5, scalar2=1.5,
                    op0=mybir.AluOpType.mult, op1=mybir.AluOpType.add,
                )
                for _ in range(2):
                    nc.vector.tensor_mul(t_, y_, y_)
                    nc.vector.tensor_mul(t_, t_, w_)
                    nc.vector.tensor_scalar(
                        out=t_, in0=t_, scalar1=-0.5, scalar2=1.5,
                        op0=mybir.AluOpType.mult, op1=mybir.AluOpType.add,
                    )
                    nc.vector.tensor_mul(y_, y_, t_)
                selr = ptiny.tile([32, 8], f32, name=f"selr{bi}", tag="selr")
                nc.vector.tensor_scalar_mul(selr[:, 0:4], mask32[:], g1[:, 5:6])
                nc.vector.tensor_scalar_mul(selr[:, 4:8], mask32[:], g1[:, 0:1])
                ps_ab = pppv.tile([128, 8], f32, name=f"ps_ab{bi}", tag="pv")
                nc.tensor.matmul(out=ps_ab[:], lhsT=sel32[:], rhs=selr[:])
                A = ptiny.tile([128, 4], f32, name=f"A{bi}", tag="A")
                Bt = ptiny.tile([128, 4], f32, name=f"Bt{bi}", tag="Bt")
                tmb = ptiny.tile([128, 4], f32, name=f"tmb{bi}", tag="tmb")
                nc.vector.tensor_mul(A[:], ps_ab[:, 0:4], gns[:])
                nc.vector.tensor_mul(tmb[:], ps_ab[:, 4:8], A[:])
                nc.vector.tensor_sub(Bt[:], gno[:], tmb[:])
                An = ptiny.tile([128, 4], f32, name=f"An{bi}", tag="An")
                Bn = ptiny.tile([128, 4], f32, name=f"Bn{bi}", tag="Bn")
                nc.vector.tensor_scalar_mul(An[:], A[:], -1.0)
                nc.vector.tensor_scalar_mul(Bn[:], Bt[:], -1.0)

                yield
                # transposed normalize: silu(x^T * A + B) -> fp8 xnT packed
                # [128, 2(c-sub), 1024] per 256-channel pair for DoubleRow;
                # channel c = 128*(2j + jj) + p lives at xnT[j][p, jj, :]
                xnT = [
                    pxnT.tile([128, 2, N], fp8, name=f"xnT{bi}_{j}", tag="xnT")
                    for j in range(2)
                ]
                for jc in range(CC):
                    for half in range(2):
                        pt = pps.tile(
                            [128, 512], f32, name=f"pt{bi}_{jc}_{half}", tag="ps512"
                        )
                        for q in range(4):
                            nt = 4 * half + q
                            nc.tensor.matmul(
                                out=pt[:, 128 * q : 128 * (q + 1)],
                                lhsT=xb[:, CH * nt + 128 * jc : CH * nt + 128 * (jc + 1)],
                                rhs=identf[:],
                                is_transpose=True,
                                start=(q == 0), stop=(q == 3),
                            )
                        u = ptiny.tile([128, 512], f32, name=f"u{bi}_{jc}_{half}", tag="u")
                        nc.vector.tensor_scalar(
                            out=u[:], in0=pt[:],
                            scalar1=A[:, jc : jc + 1], scalar2=Bt[:, jc : jc + 1],
                            op0=mybir.AluOpType.mult, op1=mybir.AluOpType.add,
                        )
                        # silu(u) = u / (1 + exp(-u))
                        sg = ptiny.tile(
                            [128, 512], f32, name=f"sg{bi}_{jc}_{half}", tag="sg"
                        )
                        nc.scalar.activation(
                            sg[:], pt[:], AF.Exp,
                            bias=Bn[:, jc : jc + 1], scale=An[:, jc : jc + 1],
                        )
                        nc.vector.tensor_scalar_add(sg[:], sg[:], 1.0)
                        nc.vector.reciprocal(sg[:], sg[:])
                        nc.vector.tensor_mul(
                            xnT[jc // 2][:, jc % 2, 512 * half : 512 * (half + 1)],
                            u[:], sg[:],
                        )
                        yield

                yield
                # QKV projections (fp8 DoubleRow, contraction 512 = 2x256):
                # q, k -> [d, n] fp8; v -> [n, 8*65] fp8 with ones columns
                qt = [pq.tile([128, N], fp8, name=f"q{bi}_{dc}", tag="q") for dc in range(CC)]
                kt = [pk.tile([128, N], fp8, name=f"k{bi}_{dc}", tag="k") for dc in range(CC)]
                for which, dst in ((0, qt), (1, kt)):
                    if which == 1:
                        yield
                    for dc in range(CC):
                        for half in range(2):
                            pp = pps.tile(
                                [128, 512], f32, name=f"pqk{bi}_{which}_{dc}_{half}",
                                tag="ps512",
                            )
                            cb = 512 * which + 128 * dc
                            for j in range(2):
                                nc.tensor.matmul(
                                    out=pp[:],
                                    lhsT=wqkv8[j][:, :, cb : cb + 128],
                                    rhs=xnT[j][:, :, 512 * half : 512 * (half + 1)],
                                    perf_mode=DR,
                                    start=(j == 0), stop=(j == 1),
                                )
                            if which == 0:
                                nc.scalar.activation(
                                    dst[dc][:, 512 * half : 512 * (half + 1)], pp[:], AF.Copy
                                )
                            else:
                                nc.vector.tensor_copy(
                                    dst[dc][:, 512 * half : 512 * (half + 1)], pp[:]
                                )
                        yield
                yield
                vt = []
                for nt in range(NT):
                    t = pv.tile([128, HEADS * 65], fp8, name=f"v{bi}_{nt}", tag="v")
                    vt.append(t)
                    nc.vector.memset(
                        t[:].rearrange("p (h x) -> p h x", h=HEADS)[:, :, 64:65], 1.0
                    )
                    pp = pps.tile([128, 512], f32, name=f"pv{bi}_{nt}", tag="ps512")
                    for j in range(2):
                        nc.tensor.matmul(
                            out=pp[:],
                            lhsT=xnT[j][:, :, 128 * nt : 128 * (nt + 1)],
                            rhs=wqkv8[j][:, :, 1024:1536],
                            perf_mode=DR,
                            start=(j == 0), stop=(j == 1),
                        )
                    nc.scalar.activation(
                        t[:].rearrange("p (h x) -> p h x", h=HEADS)[:, :, 0:64],
                        pp[:].rearrange("p (h x) -> p h x", h=HEADS),
                        AF.Copy,
                    )
                    if nt % 2 == 1:
                        yield
                yield
                s["qt"], s["kt"], s["vt"] = qt, kt, vt

            gen = emit_all()

            def pull():
                try:
                    next(gen)
                except StopIteration:
                    pass

            return [pull] * 40

        def attention(bi, extra=None):
            s = state[bi]
            qt, kt, vt = s["qt"], s["kt"], s["vt"]
            ao = pao.tile([128, NT * HIDDEN], bf16, name=f"ao{bi}", tag="ao")
            # DVE share of exp tiles per head (cycle of 4 heads)
            DVE_PATTERNS = ((2, 5), (5,), (1, 4), (2, 5))

            def emit_sim_exp(h, jt):
                dc = h // 2
                r0 = 64 * (h % 2)
                psim = ppsim.tile([128, N], f32, name=f"psim{bi}_{h}_{jt}", tag="sim")
                for half in range(2):
                    nc.tensor.matmul(
                        out=psim[:, 512 * half : 512 * (half + 1)],
                        lhsT=kt[dc][r0 : r0 + 64, 128 * jt : 128 * (jt + 1)],
                        rhs=qt[dc][r0 : r0 + 64, 512 * half : 512 * (half + 1)],
                    )
                et = pe.tile([128, N], fp8, name=f"eT{bi}_{h}_{jt}", tag="eT")
                if jt in DVE_PATTERNS[h % 4]:
                    nc.vector._custom_dve(
                        exp3, out=et[:], in0=psim[:],
                        s0=EXP_SCALE**3 / 6.0, s1=EXP_SCALE**2 / 2.0, imm2=EXP_SCALE,
                    )
                else:
                    nc.scalar.activation(et[:], psim[:], AF.Exp, scale=EXP_SCALE)
                return et

            def new_pvctx(h, eT):
                ppvs = [
                    pppv.tile([128, 4 * 65], f32, name=f"ppv{bi}_{h}_{ig}", tag="pv")
                    for ig in range(2)
                ]
                return (h, eT, ppvs)

            def emit_pv_chunk(ctx_pv, jt):
                h, eT, ppvs = ctx_pv
                for ig in range(2):
                    for ii in range(4):
                        it = 4 * ig + ii
                        nc.tensor.matmul(
                            out=ppvs[ig][:, 65 * ii : 65 * (ii + 1)],
                            lhsT=eT[jt][:, 128 * it : 128 * (it + 1)],
                            rhs=vt[jt][:, 65 * h : 65 * (h + 1)],
                            start=(jt == 0 and ii == 0),
                            stop=(jt == NT - 1 and ii == 3),
                        )

            def emit_pv_drain(ctx_pv):
                h, eT, ppvs = ctx_pv
                for ig in range(2):
                    ppv = ppvs[ig]
                    rc4 = prc.tile([128, 4], f32, name=f"rc4{bi}_{h}_{ig}", tag="rc")
                    ppv_v = ppv[:].rearrange("p (i x) -> p i x", x=65)
                    nc.vector.reciprocal(rc4[:], ppv_v[:, :, 64:65])
                    nc.vector.tensor_mul(
                        ao[:].rearrange("p (i c) -> p i c", i=NT)[
                            :, 4 * ig : 4 * ig + 4, 64 * h : 64 * (h + 1)
                        ],
                        ppv_v[:, :, 0:64],
                        rc4[:].rearrange("p (i o) -> p i o", o=1).broadcast_to(
                            [128, 4, 64]
                        ),
                    )

            # 1-head software pipeline, interleaved at j-tile granularity
            pvctx = None
            for h in range(HEADS):
                for f in (extra[h::HEADS] if extra else ()):
                    f()
                eT = []
                for jt in range(NT):
                    eT.append(emit_sim_exp(h, jt))
                    if pvctx is not None:
                        emit_pv_chunk(pvctx, jt)
                if pvctx is not None:
                    emit_pv_drain(pvctx)
                pvctx = new_pvctx(h, eT)
            for jt in range(NT):
                emit_pv_chunk(pvctx, jt)
            emit_pv_drain(pvctx)
            s["ao"] = ao

        def make_epilogue_chunks(bi, b):
            s = state[bi]
            xb, ao = s["xb"], s["ao"]
            chunks = []
            aoT = [
                paoT.tile([128, N], bf16, name=f"aoT{bi}_{dc}", tag="aoT")
                for dc in range(CC)
            ]

            def aot_chunk(dc2):
                for half in range(2):
                    pt2 = pps.tile(
                        [128, 512], bf16, name=f"pt2{bi}_{dc2}_{half}", tag="ps512"
                    )
                    for q in range(4):
                        nt = 4 * half + q
                        nc.tensor.matmul(
                            out=pt2[:, 128 * q : 128 * (q + 1)],
                            lhsT=ao[
                                :, HIDDEN * nt + 128 * dc2 : HIDDEN * nt + 128 * (dc2 + 1)
                            ],
                            rhs=identb[:],
                            is_transpose=True,
                            start=(q == 0), stop=(q == 3),
                        )
                    nc.scalar.activation(
                        aoT[dc2][:, 512 * half : 512 * (half + 1)], pt2[:], AF.Copy
                    )

            for dc2 in range(CC):
                chunks.append(lambda dc2=dc2: aot_chunk(dc2))
            ob = pout.tile([128, NT * CH], f32, name=f"ob{bi}", tag="ob")

            def oproj_chunk(g):
                for nt in (2 * g, 2 * g + 1):
                    pf = pps.tile([128, CH], f32, name=f"pf{bi}_{nt}", tag="ps512")
                    for dc2 in range(CC):
                        nc.tensor.matmul(
                            out=pf[:],
                            lhsT=aoT[dc2][:, 128 * nt : 128 * (nt + 1)],
                            rhs=wout[dc2][:],
                            start=(dc2 == 0), stop=(dc2 == CC - 1),
                        )
                    nc.vector.tensor_add(
                        ob[:, CH * nt : CH * (nt + 1)], pf[:],
                        xb[:, CH * nt : CH * (nt + 1)],
                    )
                    nc.vector.tensor_add(
                        ob[:, CH * nt : CH * (nt + 1)],
                        ob[:, CH * nt : CH * (nt + 1)], bb[:],
                    )
                nc.sync.dma_start(
                    out=out_d[b, 256 * g : 256 * (g + 1), :].rearrange(
                        "(t p) c -> p t c", p=128
                    ),
                    in_=ob[:, 2 * CH * g : 2 * CH * (g + 1)].rearrange(
                        "p (t c) -> p t c", t=2
                    ),
                )

            for g in range(4):
                chunks.append(lambda g=g: oproj_chunk(g))
            return chunks

        # cross-group software pipeline: batch b's prologue and the previous
        # batch's epilogue interleave into the current attention, so in a
        # repeat loop (and in the 4-batch steady state generally) no engine
        # drains between groups
        epi_prev = None
        for g in range(repeat):
            b0, b1 = 2 * g, 2 * g + 1
            if g == 0:
                for f in make_prologue_chunks(b0, 0):
                    f()
            emit_xload(b1, 1)
            extra0 = make_prologue_chunks(b1, 1) + (epi_prev or [])
            attention(b0, extra=extra0)
            extra1 = make_epilogue_chunks(b0, 0)
            del state[b0]
            if g + 1 < repeat:
                emit_xload(b0 + 2, 0)
                extra1 = extra1 + make_prologue_chunks(b0 + 2, 0)
            attention(b1, extra=extra1)
            epi_prev = make_epilogue_chunks(b1, 1)
            del state[b1]
        for f in epi_prev:
            f()
        if tout_d is not None:
            tt = pc.tile([128, 16], f32, name="tt", tag="tt")
            nc.vector.memset(tt[:], 1.0)
            nc.sync.dma_start(out=tout_d[:, :], in_=tt[:])

    nc.compile()
    return nc


def make_in_maps(x, gn_scale, gn_offset, w_qkv, w_out, b_out):
    import ml_dtypes

    bf16 = ml_dtypes.bfloat16
    fp8 = ml_dtypes.float8_e4m3
    x = np.asarray(x, dtype=np.float32)
    gn_scale = np.asarray(gn_scale, dtype=np.float32)
    gn_offset = np.asarray(gn_offset, dtype=np.float32)
    w_qkv = np.asarray(w_qkv, dtype=np.float32)
    w_out = np.asarray(w_out, dtype=np.float32)
    b_out = np.asarray(b_out, dtype=np.float32)

    # 16x scaling into fp8; channel c = 128*(2j+jj)+p -> wqkv8[j, p, jj, :]
    w16 = (W_SCALE * w_qkv).reshape(2, 2, 128, 3 * HIDDEN).transpose(0, 2, 1, 3)
    wqkv8 = np.ascontiguousarray(w16.astype(fp8))

    wout_h = np.ascontiguousarray((w_out / W_SCALE).astype(bf16))
    identf = np.eye(128, dtype=np.float32)
    identb = np.eye(128, dtype=np.float32).astype(bf16)
    g_idx = np.arange(32)
    sel32 = (g_idx[:, None] % 8 == np.arange(128)[None, :] // 16).astype(np.float32)
    mask32 = (g_idx[:, None] // 8 == np.arange(4)[None, :]).astype(np.float32)
    gns = np.ascontiguousarray(gn_scale.reshape(4, 128).T.astype(np.float32))
    gno = np.ascontiguousarray(gn_offset.reshape(4, 128).T.astype(np.float32))
    bb = np.broadcast_to(b_out, (128, CH)).copy()
    ones = np.ones((128, 1), dtype=np.float32)

    xr = x.reshape(B, N, CH)
    in_maps = []
    for i in range(N_CORES):
        in_maps.append(
            {
                "x": np.ascontiguousarray(xr[BPC * i : BPC * (i + 1)]),
                "wqkv8": wqkv8,
                "wout": wout_h,
                "identf": identf,
                "identb": identb,
                "sel32": sel32,
                "mask32": mask32,
                "gns": gns,
                "gno": gno,
                "bb": bb,
                "ones": ones,
            }
        )
    return in_maps


_NC_CACHE = None


def kernel(x, gn_scale, gn_offset, w_qkv, w_out, b_out, _return_extra=False):
    global _NC_CACHE
    from concourse.bass_utils import run_bass_kernel_spmd

    if _NC_CACHE is None:
        _NC_CACHE = build_program()
    nc = _NC_CACHE
    in_maps = make_in_maps(x, gn_scale, gn_offset, w_qkv, w_out, b_out)
    res = run_bass_kernel_spmd(nc, in_maps, list(range(N_CORES)))
    outs = [res.results[i]["out"] for i in range(N_CORES)]
    out = np.concatenate(outs, axis=0).reshape(B, HGT, WID, CH).astype(np.float32)
    if _return_extra:
        return out, res
    return out
